# revision 13
# baseline (speedup 1.0000x reference)
"""MultiHeadAttention Trainium2 kernel (8 NeuronCores).

Problem: b=2, n=2048, dim=1024, heads=16, dim_head=64, causal attention,
padding mask (all-ones in this problem), fp32 I/O.

Sharding (per core c in 0..7): batch b = c//4, head-group g = c%4 (4 heads).
  - attention is fully local per (batch, head-group)
  - attnout^T (bf16) is AllGathered inside each 4-core batch group, split
    into four 512-query chunks so the collectives overlap attention compute
  - each core then computes a disjoint 256-column slice of the output
    projection (Wo column split), so host reassembly is pure concatenation.

Device layout notes:
  - host passes x[b] TRANSPOSED (xT [1024, 2048]) so the contraction dim of
    every projection matmul is on partitions; no on-device transposes at all.
  - S is computed transposed (S^T [keys, q]) so that P^T = exp(S^T) is
    directly the moving operand of the AV matmul.
  - softmax runs without max subtraction: logits are ~N(0,1) (|S| < ~12 for
    these inputs), exp is safe in fp32.
  - row-sums of exp come from a ones-column appended to V (65th column), so
    no partition reductions are needed.
  - all matmuls are bf16 (f32r measured ~2x slower on HW and kept the PE
    clock throttled).
  - the two heads of a head-pair share one [128, 1024] S^T PSUM tile, are
    computed by concurrently-running row-tiled matmuls (contraction is only
    64), and share a single merged exp ACTIVATE to amortize the ~350ns ACT
    fixed overhead.
"""

import numpy as np

B = 2
N = 2048
DIM = 1024
HEADS = 16
DIM_HEAD = 64
SCALE = DIM_HEAD**-0.5  # 0.125
NCORES = 8
GROUPS = 4  # head groups (cores per batch)
GDIM = DIM // GROUPS  # 256 features per core
P = 128
QB = 512  # query macroblock
NB = N // QB  # 4 q-macroblocks
KO = DIM // P  # 8 contraction chunks
JT = N // P  # 16 key tiles

_cached = None


def _build_nc():
    import concourse.mybir as mybir
    import concourse.tile as tile
    from concourse import bacc

    f32 = mybir.dt.float32
    bf16 = mybir.dt.bfloat16
    Exp = mybir.ActivationFunctionType.Exp

    nc = bacc.Bacc(num_devices=NCORES)

    xT = nc.dram_tensor("xT", [DIM, N], f32, kind="ExternalInput")
    wq = nc.dram_tensor("wq", [DIM, GDIM], f32, kind="ExternalInput")
    wk = nc.dram_tensor("wk", [DIM, GDIM], f32, kind="ExternalInput")
    wv = nc.dram_tensor("wv", [DIM, GDIM], f32, kind="ExternalInput")
    wo = nc.dram_tensor("wo", [DIM, GDIM], f32, kind="ExternalInput")
    outT = nc.dram_tensor("outT", [GDIM, N], f32, kind="ExternalOutput")

    with tile.TileContext(nc) as tc:
        with (
            tc.tile_pool(name="stage", bufs=3) as stage,    # fp32 staging
            tc.tile_pool(name="io", bufs=KO) as io,         # xc / ag chunks
            tc.tile_pool(name="wpool", bufs=1) as wpool,    # weights
            tc.tile_pool(name="qkpool", bufs=1) as qkpool,  # QT/KT/V
            tc.tile_pool(name="ptpool", bufs=6) as ptpool,  # exp(S^T)
            tc.tile_pool(name="work", bufs=4) as work,      # small staging
            tc.tile_pool(name="psS", bufs=2, space="PSUM") as psS,  # 2x2 banks
            tc.tile_pool(name="psO", bufs=4, space="PSUM") as psO,  # 4x1 banks
            tc.tile_pool(name="dram", bufs=1, space="DRAM") as dram,
        ):
            # ---- load weights fp32, cast to bf16 ----
            w_bf = {}
            for idx, (nm, t_dram) in enumerate(
                (("wq", wq), ("wk", wk), ("wv", wv), ("wo", wo))
            ):
                st = stage.tile([P, KO, GDIM], f32, tag="stage", name=f"st_{nm}")
                nc.sync.dma_start(st[:], t_dram.rearrange("(ko p) f -> p ko f", p=P))
                wbf = wpool.tile([P, KO, GDIM], bf16, name=f"bf_{nm}")
                if idx % 2 == 0:
                    nc.vector.tensor_copy(wbf[:], st[:])
                else:
                    nc.scalar.copy(wbf[:], st[:])
                w_bf[nm] = wbf
            wq_bf, wk_bf, wv_bf, wo_bf = w_bf["wq"], w_bf["wk"], w_bf["wv"], w_bf["wo"]

            # ---- load xT chunks fp32, cast to bf16 ----
            xc = []
            for k in range(KO):
                st = stage.tile([P, N], f32, tag="stage", name=f"stx{k}")
                nc.sync.dma_start(st[:], xT[k * P : (k + 1) * P, :])
                xck = io.tile([P, N], bf16, tag="io", name=f"xc{k}")
                if k % 3 == 0:
                    nc.vector.tensor_copy(xck[:], st[:])
                elif k % 3 == 1:
                    nc.scalar.copy(xck[:], st[:])
                else:
                    nc.gpsimd.tensor_copy(xck[:], st[:])
                xc.append(xck)


            # ---- Q^T, K^T projections -> bf16 [128f, 2, 2048] (Q pre-scaled) --
            QT = qkpool.tile([P, 2, N], bf16)
            KT = qkpool.tile([P, 2, N], bf16)
            for ni in range(NB):
                nsl = slice(ni * QB, (ni + 1) * QB)
                for fi in range(2):
                    pq = psS.tile([P, 1024], f32, tag="S", name="pq")
                    for k in range(KO):
                        nc.tensor.matmul(
                            pq[:, :QB],
                            wq_bf[:, k, fi * P : (fi + 1) * P],
                            xc[k][:, nsl],
                            start=(k == 0),
                            stop=(k == KO - 1),
                        )
                    # copy out with the softmax scale folded into Q
                    nc.scalar.mul(QT[:, fi, nsl], pq[:, :QB], SCALE)
                    pk = psS.tile([P, 1024], f32, tag="S", name="pk")
                    for k in range(KO):
                        nc.tensor.matmul(
                            pk[:, :QB],
                            wk_bf[:, k, fi * P : (fi + 1) * P],
                            xc[k][:, nsl],
                            start=(k == 0),
                            stop=(k == KO - 1),
                        )
                    nc.vector.tensor_copy(KT[:, fi, nsl], pk[:, :QB])

            # ---- V (natural layout) + ones column: [128j, 16, 4, 65] bf16 ----
            V_sb = qkpool.tile([P, JT, GROUPS, DIM_HEAD + 1], bf16)
            nc.vector.memset(V_sb[:, :, :, DIM_HEAD : DIM_HEAD + 1], 1.0)
            for jt in range(JT):
                pvt = psS.tile([P, 1024], f32, tag="S", name="pv")
                pv = pvt[:, :GDIM]
                for k in range(KO):
                    nc.tensor.matmul(
                        pv,
                        xc[k][:, jt * P : (jt + 1) * P],
                        wv_bf[:, k, :],
                        start=(k == 0),
                        stop=(k == KO - 1),
                    )
                nc.vector.tensor_copy(
                    V_sb[:, jt, :, 0:DIM_HEAD],
                    pv.rearrange("p (h d) -> p h d", h=GROUPS),
                )

            # ---- attention (mb-outer so AllGather chunks overlap compute) ----
            ag_outs = []
            for mb in range(NB):
                qsl = slice(mb * QB, (mb + 1) * QB)
                njc = 4 * (mb + 1)
                # one [65, 512] accumulator per head
                po = [
                    psO.tile([DIM_HEAD + 1, QB], f32, tag="O", name=f"po{h}")
                    for h in range(GROUPS)
                ]
                for jc in range(njc):
                    jsl = slice(jc * P, (jc + 1) * P)
                    t = jc - 4 * mb  # >= 0 on the diagonal 512-block
                    for hp in range(2):
                        ps = psS.tile([P, 1024], f32, tag="S", name=f"ps{hp}")
                        for s in range(2):
                            prow = slice(64 * s, 64 * s + 64)
                            nc.tensor.matmul(
                                ps[:, s * QB : (s + 1) * QB],
                                KT[prow, hp, jsl],
                                QT[prow, hp, qsl],
                                tile_position=(64 * s, 0),
                            )
                        pt = ptpool.tile([P, 1024], bf16, tag="pt", name="pt")
                        if t < 0:
                            nc.scalar.activation(pt[:], ps[:], Exp)
                        else:
                            c0 = t * P
                            ps3 = ps.rearrange("p (s q) -> p s q", s=2)
                            pt3 = pt.rearrange("p (s q) -> p s q", s=2)
                            if c0 > 0:
                                nc.vector.memset(pt3[:, :, :c0], 0.0)
                            nc.scalar.activation(pt3[:, :, c0:], ps3[:, :, c0:], Exp)
                            # causal: keep iff (q - j) >= 0  <=>  i1 - r >= 0
                            nc.gpsimd.affine_select(
                                out=pt3[:, :, c0:],
                                in_=pt3[:, :, c0:],
                                pattern=[[0, 2], [1, QB - c0]],
                                compare_op=mybir.AluOpType.is_ge,
                                fill=0.0,
                                base=0,
                                channel_multiplier=-1,
                            )
                        for s in range(2):
                            head = 2 * hp + s
                            nc.tensor.matmul(
                                po[head][:],
                                V_sb[:, jc, head, :],
                                pt[:, s * QB : (s + 1) * QB],
                                start=(jc == 0),
                                stop=(jc == njc - 1),
                            )
                # normalize + emit this q-chunk's AllGather
                attnT_mb = work.tile([DIM_HEAD, GROUPS, QB], bf16, tag="attnT")
                for head in range(GROUPS):
                    recip = work.tile([1, QB], f32, tag="recip", name="recip")
                    nc.vector.reciprocal(
                        recip[:], po[head][DIM_HEAD : DIM_HEAD + 1, :]
                    )
                    bc = work.tile([DIM_HEAD, QB], f32, tag="bc", name="bc")
                    nc.gpsimd.partition_broadcast(bc[:], recip[:])
                    nc.vector.tensor_mul(
                        attnT_mb[:, head, :], po[head][0:DIM_HEAD, :], bc[:]
                    )
                ag_in = dram.tile(
                    [GDIM, QB], bf16, name=f"ag_in{mb}", tag=f"ag_in{mb}"
                )
                ag_out = dram.tile(
                    [DIM, QB], bf16, name=f"ag_out{mb}", tag=f"ag_out{mb}"
                )
                nc.sync.dma_start(
                    ag_in.rearrange("(h p) q -> p h q", p=DIM_HEAD), attnT_mb[:]
                )
                nc.gpsimd.collective_compute(
                    "AllGather",
                    mybir.AluOpType.bypass,
                    replica_groups=[[0, 1, 2, 3], [4, 5, 6, 7]],
                    ins=[ag_in.opt()],
                    outs=[ag_out.opt()],
                )
                ag_outs.append(ag_out)

            # ---- output projection (Wo column slice), per q-chunk ----
            for mb in range(NB):
                qsl = slice(mb * QB, (mb + 1) * QB)
                agb = []
                for k in range(KO):
                    agbk = io.tile([P, N], bf16, tag="io", name=f"agb{k}")
                    nc.sync.dma_start(
                        agbk[:, :QB], ag_outs[mb][k * P : (k + 1) * P, :]
                    )
                    agb.append(agbk)
                for fi in range(2):
                    pw = psS.tile([P, 1024], f32, tag="S", name="pw")
                    for k in range(KO):
                        nc.tensor.matmul(
                            pw[:, :QB],
                            wo_bf[:, k, fi * P : (fi + 1) * P],
                            agb[k][:, :QB],
                            start=(k == 0),
                            stop=(k == KO - 1),
                        )
                    ot = work.tile([P, QB], f32, tag="ot", name="ot")
                    nc.vector.tensor_copy(ot[:], pw[:, :QB])
                    nc.sync.dma_start(outT[fi * P : (fi + 1) * P, qsl], ot[:])

    nc.finalize()
    return nc


def _get_nc():
    global _cached
    if _cached is None:
        _cached = _build_nc()
    return _cached


def _m0_const():
    import ml_dtypes

    m = (np.arange(QB)[None, :] >= np.arange(P)[:, None]).astype(np.float32)
    return m.astype(ml_dtypes.bfloat16)


def kernel(x, mask, Wq, Wk, Wv, Wo):
    x = np.asarray(x, dtype=np.float32)
    mask = np.asarray(mask)
    Wq = np.asarray(Wq, dtype=np.float32)
    Wk = np.asarray(Wk, dtype=np.float32)
    Wv = np.asarray(Wv, dtype=np.float32)
    Wo = np.asarray(Wo, dtype=np.float32)
    # this problem's padding mask is all-True (spec fill: ones); the kernel
    # relies on that (only the causal mask is applied on device).
    assert mask.all(), "kernel specialized for all-ones padding mask"

    from concourse import bass_utils

    nc = _get_nc()

    xTs = [np.ascontiguousarray(x[b].T) for b in range(B)]
    in_maps = []
    for c in range(NCORES):
        b, g = divmod(c, GROUPS)
        gsl = slice(g * GDIM, (g + 1) * GDIM)
        in_maps.append(
            {
                "xT": xTs[b],
                "wq": np.ascontiguousarray(Wq[:, gsl]),
                "wk": np.ascontiguousarray(Wk[:, gsl]),
                "wv": np.ascontiguousarray(Wv[:, gsl]),
                "wo": np.ascontiguousarray(Wo[:, gsl]),
            }
        )

    res = bass_utils.run_bass_kernel_spmd(nc, in_maps, core_ids=list(range(NCORES)))

    out = np.empty((B, N, DIM), dtype=np.float32)
    for c in range(NCORES):
        b, g = divmod(c, GROUPS)
        out[b, :, g * GDIM : (g + 1) * GDIM] = res.results[c]["outT"].T
    return out


# revision 14
# speedup vs baseline: 1.1040x; 1.1040x over previous
"""MultiHeadAttention Trainium2 kernel (8 NeuronCores).

Problem: b=2, n=2048, dim=1024, heads=16, dim_head=64, causal attention,
padding mask (all-ones in this problem), fp32 I/O.

Sharding (per core c in 0..7): batch b = c//4, head-group g = c%4 (4 heads).
  - attention is fully local per (batch, head-group)
  - attnout^T (bf16) is AllGathered inside each 4-core batch group, split
    into four 512-query chunks so the collectives overlap attention compute
  - each core then computes a disjoint 256-column slice of the output
    projection (Wo column split), so host reassembly is pure concatenation.

Device layout notes:
  - host passes x[b] TRANSPOSED (xT [1024, 2048]) so the contraction dim of
    every projection matmul is on partitions; no on-device transposes at all.
  - S is computed transposed (S^T [keys, q]) so that P^T = exp(S^T) is
    directly the moving operand of the AV matmul.
  - softmax runs without max subtraction: logits are ~N(0,1) (|S| < ~12 for
    these inputs), exp is safe in fp32.
  - row-sums of exp come from a ones-column appended to V (65th column), so
    no partition reductions are needed.
  - all matmuls are bf16 (f32r measured ~2x slower on HW and kept the PE
    clock throttled).
  - the two heads of a head-pair share one [128, 1024] S^T PSUM tile, are
    computed by concurrently-running row-tiled matmuls (contraction is only
    64), and share a single merged exp ACTIVATE to amortize the ~350ns ACT
    fixed overhead.
"""

import numpy as np

B = 2
N = 2048
DIM = 1024
HEADS = 16
DIM_HEAD = 64
SCALE = DIM_HEAD**-0.5  # 0.125
NCORES = 8
GROUPS = 4  # head groups (cores per batch)
GDIM = DIM // GROUPS  # 256 features per core
P = 128
QB = 512  # query macroblock
NB = N // QB  # 4 q-macroblocks
KO = DIM // P  # 8 contraction chunks
JT = N // P  # 16 key tiles

_cached = None


def _build_nc():
    import concourse.mybir as mybir
    import concourse.tile as tile
    from concourse import bacc

    f32 = mybir.dt.float32
    bf16 = mybir.dt.bfloat16
    Exp = mybir.ActivationFunctionType.Exp

    nc = bacc.Bacc(num_devices=NCORES)

    xT = nc.dram_tensor("xT", [DIM, N], f32, kind="ExternalInput")
    wq = nc.dram_tensor("wq", [DIM, GDIM], f32, kind="ExternalInput")
    wk = nc.dram_tensor("wk", [DIM, GDIM], f32, kind="ExternalInput")
    wv = nc.dram_tensor("wv", [DIM, GDIM], f32, kind="ExternalInput")
    wo = nc.dram_tensor("wo", [DIM, GDIM], f32, kind="ExternalInput")
    outT = nc.dram_tensor("outT", [GDIM, N], f32, kind="ExternalOutput")

    with tile.TileContext(nc) as tc:
        with (
            tc.tile_pool(name="stage", bufs=3) as stage,    # fp32 staging
            tc.tile_pool(name="io", bufs=KO) as io,         # xc / ag chunks
            tc.tile_pool(name="wpool", bufs=1) as wpool,    # weights
            tc.tile_pool(name="qkpool", bufs=1) as qkpool,  # QT/KT/V
            tc.tile_pool(name="ptpool", bufs=6) as ptpool,  # exp(S^T)
            tc.tile_pool(name="work", bufs=4) as work,      # small staging
            tc.tile_pool(name="psS", bufs=2, space="PSUM") as psS,  # 2x2 banks
            tc.tile_pool(name="psO", bufs=4, space="PSUM") as psO,  # 4x1 banks
            tc.tile_pool(name="dram", bufs=1, space="DRAM") as dram,
        ):
            # ---- load weights fp32, cast to bf16 ----
            w_bf = {}
            for idx, (nm, t_dram) in enumerate(
                (("wq", wq), ("wk", wk), ("wv", wv), ("wo", wo))
            ):
                st = stage.tile([P, KO, GDIM], f32, tag="stage", name=f"st_{nm}")
                nc.sync.dma_start(st[:], t_dram.rearrange("(ko p) f -> p ko f", p=P))
                wbf = wpool.tile([P, KO, GDIM], bf16, name=f"bf_{nm}")
                if idx % 2 == 0:
                    nc.vector.tensor_copy(wbf[:], st[:])
                else:
                    nc.scalar.copy(wbf[:], st[:])
                w_bf[nm] = wbf
            wq_bf, wk_bf, wv_bf, wo_bf = w_bf["wq"], w_bf["wk"], w_bf["wv"], w_bf["wo"]

            # ---- load xT chunks fp32, cast to bf16 ----
            xc = []
            for k in range(KO):
                st = stage.tile([P, N], f32, tag="stage", name=f"stx{k}")
                nc.sync.dma_start(st[:], xT[k * P : (k + 1) * P, :])
                xck = io.tile([P, N], bf16, tag="io", name=f"xc{k}")
                if k % 3 == 0:
                    nc.vector.tensor_copy(xck[:], st[:])
                elif k % 3 == 1:
                    nc.scalar.copy(xck[:], st[:])
                else:
                    nc.gpsimd.tensor_copy(xck[:], st[:])
                xc.append(xck)


            # ---- Q^T, K^T projections -> bf16 [128f, 2, 2048] (Q pre-scaled) --
            QT = qkpool.tile([P, 2, N], bf16)
            KT = qkpool.tile([P, 2, N], bf16)
            for ni in range(NB):
                nsl = slice(ni * QB, (ni + 1) * QB)
                for fi in range(2):
                    pq = psS.tile([P, 1024], f32, tag="S", name="pq")
                    for k in range(KO):
                        nc.tensor.matmul(
                            pq[:, :QB],
                            wq_bf[:, k, fi * P : (fi + 1) * P],
                            xc[k][:, nsl],
                            start=(k == 0),
                            stop=(k == KO - 1),
                        )
                    # copy out with the softmax scale folded into Q
                    nc.scalar.mul(QT[:, fi, nsl], pq[:, :QB], SCALE)
                    pk = psS.tile([P, 1024], f32, tag="S", name="pk")
                    for k in range(KO):
                        nc.tensor.matmul(
                            pk[:, :QB],
                            wk_bf[:, k, fi * P : (fi + 1) * P],
                            xc[k][:, nsl],
                            start=(k == 0),
                            stop=(k == KO - 1),
                        )
                    nc.vector.tensor_copy(KT[:, fi, nsl], pk[:, :QB])

            # ---- V (natural layout) + ones column: [128j, 16, 4, 65] bf16 ----
            V_sb = qkpool.tile([P, JT, GROUPS, DIM_HEAD + 1], bf16)
            nc.vector.memset(V_sb[:, :, :, DIM_HEAD : DIM_HEAD + 1], 1.0)
            for jt in range(JT):
                pvt = psS.tile([P, 1024], f32, tag="S", name="pv")
                pv = pvt[:, :GDIM]
                for k in range(KO):
                    nc.tensor.matmul(
                        pv,
                        xc[k][:, jt * P : (jt + 1) * P],
                        wv_bf[:, k, :],
                        start=(k == 0),
                        stop=(k == KO - 1),
                    )
                nc.vector.tensor_copy(
                    V_sb[:, jt, :, 0:DIM_HEAD],
                    pv.rearrange("p (h d) -> p h d", h=GROUPS),
                )

            # ---- attention (mb-outer so AllGather chunks overlap compute) ----
            ag_outs = []
            for mb in range(NB):
                qsl = slice(mb * QB, (mb + 1) * QB)
                njc = 4 * (mb + 1)
                # one [65, 512] accumulator per head
                po = [
                    psO.tile([DIM_HEAD + 1, QB], f32, tag="O", name=f"po{h}")
                    for h in range(GROUPS)
                ]
                for jc in range(njc):
                    jsl = slice(jc * P, (jc + 1) * P)
                    t = jc - 4 * mb  # >= 0 on the diagonal 512-block
                    for hp in range(2):
                        ps = psS.tile([P, 1024], f32, tag="S", name=f"ps{hp}")
                        for s in range(2):
                            prow = slice(64 * s, 64 * s + 64)
                            nc.tensor.matmul(
                                ps[:, s * QB : (s + 1) * QB],
                                KT[prow, hp, jsl],
                                QT[prow, hp, qsl],
                                tile_position=(64 * s, 0),
                            )
                        pt = ptpool.tile([P, 1024], bf16, tag="pt", name="pt")
                        if t < 0:
                            nc.scalar.activation(pt[:], ps[:], Exp)
                        else:
                            c0 = t * P
                            ps3 = ps.rearrange("p (s q) -> p s q", s=2)
                            pt3 = pt.rearrange("p (s q) -> p s q", s=2)
                            if c0 > 0:
                                nc.vector.memset(pt3[:, :, :c0], 0.0)
                            nc.scalar.activation(pt3[:, :, c0:], ps3[:, :, c0:], Exp)
                            # causal: keep iff (q - j) >= 0  <=>  i1 - r >= 0
                            nc.gpsimd.affine_select(
                                out=pt3[:, :, c0:],
                                in_=pt3[:, :, c0:],
                                pattern=[[0, 2], [1, QB - c0]],
                                compare_op=mybir.AluOpType.is_ge,
                                fill=0.0,
                                base=0,
                                channel_multiplier=-1,
                            )
                        for s in range(2):
                            head = 2 * hp + s
                            nc.tensor.matmul(
                                po[head][:],
                                V_sb[:, jc, head, :],
                                pt[:, s * QB : (s + 1) * QB],
                                start=(jc == 0),
                                stop=(jc == njc - 1),
                            )
                # normalize + emit this q-chunk's AllGather
                attnT_mb = work.tile([DIM_HEAD, GROUPS, QB], bf16, tag="attnT")
                for head in range(GROUPS):
                    # 1/s = exp(-ln(s)) on ScalarE (same act table set as the
                    # attention exp; DVE reciprocal costs 3.3us per call)
                    lntmp = work.tile([1, QB], f32, tag="lntmp", name="lntmp")
                    nc.scalar.activation(
                        lntmp[:],
                        po[head][DIM_HEAD : DIM_HEAD + 1, :],
                        mybir.ActivationFunctionType.Ln,
                    )
                    recip = work.tile([1, QB], f32, tag="recip", name="recip")
                    nc.scalar.activation(
                        recip[:], lntmp[:], mybir.ActivationFunctionType.Exp,
                        scale=-1.0,
                    )
                    bc = work.tile([DIM_HEAD, QB], f32, tag="bc", name="bc")
                    nc.gpsimd.partition_broadcast(bc[:], recip[:])
                    nc.vector.tensor_mul(
                        attnT_mb[:, head, :], po[head][0:DIM_HEAD, :], bc[:]
                    )
                ag_in = dram.tile(
                    [GDIM, QB], bf16, name=f"ag_in{mb}", tag=f"ag_in{mb}"
                )
                ag_out = dram.tile(
                    [DIM, QB], bf16, name=f"ag_out{mb}", tag=f"ag_out{mb}"
                )
                nc.sync.dma_start(
                    ag_in.rearrange("(h p) q -> p h q", p=DIM_HEAD), attnT_mb[:]
                )
                nc.gpsimd.collective_compute(
                    "AllGather",
                    mybir.AluOpType.bypass,
                    replica_groups=[[0, 1, 2, 3], [4, 5, 6, 7]],
                    ins=[ag_in.opt()],
                    outs=[ag_out.opt()],
                )
                ag_outs.append(ag_out)

            # ---- output projection (Wo column slice), per q-chunk ----
            for mb in range(NB):
                qsl = slice(mb * QB, (mb + 1) * QB)
                agb = []
                for k in range(KO):
                    agbk = io.tile([P, N], bf16, tag="io", name=f"agb{k}")
                    nc.sync.dma_start(
                        agbk[:, :QB], ag_outs[mb][k * P : (k + 1) * P, :]
                    )
                    agb.append(agbk)
                for fi in range(2):
                    pw = psS.tile([P, 1024], f32, tag="S", name="pw")
                    for k in range(KO):
                        nc.tensor.matmul(
                            pw[:, :QB],
                            wo_bf[:, k, fi * P : (fi + 1) * P],
                            agb[k][:, :QB],
                            start=(k == 0),
                            stop=(k == KO - 1),
                        )
                    ot = work.tile([P, QB], f32, tag="ot", name="ot")
                    nc.vector.tensor_copy(ot[:], pw[:, :QB])
                    nc.sync.dma_start(outT[fi * P : (fi + 1) * P, qsl], ot[:])

    nc.finalize()
    return nc


def _get_nc():
    global _cached
    if _cached is None:
        _cached = _build_nc()
    return _cached


def _m0_const():
    import ml_dtypes

    m = (np.arange(QB)[None, :] >= np.arange(P)[:, None]).astype(np.float32)
    return m.astype(ml_dtypes.bfloat16)


def kernel(x, mask, Wq, Wk, Wv, Wo):
    x = np.asarray(x, dtype=np.float32)
    mask = np.asarray(mask)
    Wq = np.asarray(Wq, dtype=np.float32)
    Wk = np.asarray(Wk, dtype=np.float32)
    Wv = np.asarray(Wv, dtype=np.float32)
    Wo = np.asarray(Wo, dtype=np.float32)
    # this problem's padding mask is all-True (spec fill: ones); the kernel
    # relies on that (only the causal mask is applied on device).
    assert mask.all(), "kernel specialized for all-ones padding mask"

    from concourse import bass_utils

    nc = _get_nc()

    xTs = [np.ascontiguousarray(x[b].T) for b in range(B)]
    in_maps = []
    for c in range(NCORES):
        b, g = divmod(c, GROUPS)
        gsl = slice(g * GDIM, (g + 1) * GDIM)
        in_maps.append(
            {
                "xT": xTs[b],
                "wq": np.ascontiguousarray(Wq[:, gsl]),
                "wk": np.ascontiguousarray(Wk[:, gsl]),
                "wv": np.ascontiguousarray(Wv[:, gsl]),
                "wo": np.ascontiguousarray(Wo[:, gsl]),
            }
        )

    res = bass_utils.run_bass_kernel_spmd(nc, in_maps, core_ids=list(range(NCORES)))

    out = np.empty((B, N, DIM), dtype=np.float32)
    for c in range(NCORES):
        b, g = divmod(c, GROUPS)
        out[b, :, g * GDIM : (g + 1) * GDIM] = res.results[c]["outT"].T
    return out


# revision 15
# speedup vs baseline: 1.2233x; 1.1081x over previous
"""MultiHeadAttention Trainium2 kernel (8 NeuronCores).

Problem: b=2, n=2048, dim=1024, heads=16, dim_head=64, causal attention,
padding mask (all-ones in this problem), fp32 I/O.

Sharding (per core c in 0..7): batch b = c//4, head-group g = c%4 (4 heads).
  - attention is fully local per (batch, head-group)
  - attnout^T (bf16) is AllGathered inside each 4-core batch group, split
    into four 512-query chunks so the collectives overlap attention compute
  - each core then computes a disjoint 256-column slice of the output
    projection (Wo column split), so host reassembly is pure concatenation.

Device layout notes:
  - host passes x[b] TRANSPOSED (xT [1024, 2048]) so the contraction dim of
    every projection matmul is on partitions; no on-device transposes at all.
  - S is computed transposed (S^T [keys, q]) so that P^T = exp(S^T) is
    directly the moving operand of the AV matmul.
  - softmax runs without max subtraction: logits are ~N(0,1) (|S| < ~12 for
    these inputs), exp is safe in fp32.
  - row-sums of exp come from a ones-column appended to V (65th column), so
    no partition reductions are needed.
  - all matmuls are bf16 (f32r measured ~2x slower on HW and kept the PE
    clock throttled).
  - the two heads of a head-pair share one [128, 1024] S^T PSUM tile, are
    computed by concurrently-running row-tiled matmuls (contraction is only
    64), and share a single merged exp ACTIVATE to amortize the ~350ns ACT
    fixed overhead.
"""

import numpy as np

B = 2
N = 2048
DIM = 1024
HEADS = 16
DIM_HEAD = 64
SCALE = DIM_HEAD**-0.5  # 0.125
NCORES = 8
GROUPS = 4  # head groups (cores per batch)
GDIM = DIM // GROUPS  # 256 features per core
P = 128
QB = 512  # query macroblock
NB = N // QB  # 4 q-macroblocks
KO = DIM // P  # 8 contraction chunks
JT = N // P  # 16 key tiles

_cached = None


def _build_nc():
    import concourse.mybir as mybir
    import concourse.tile as tile
    from concourse import bacc

    f32 = mybir.dt.float32
    bf16 = mybir.dt.bfloat16
    Exp = mybir.ActivationFunctionType.Exp

    nc = bacc.Bacc(num_devices=NCORES)

    # We use both Exp (attention softmax) and Ln (reciprocal via exp(-ln s)).
    # The greedy table-set picker would thrash between exp_and_others and a
    # ln-only set (~2.7us per ACT_TABLE_LOAD, one per normalize). Steer it to
    # the combined set by hiding Exp/Ln from every other set. The dict is
    # functools.cache'd and keyed by set NAME with stable insertion order, so
    # mutating entries in place keeps act_func_set_id assignment correct.
    from concourse import hw_specs

    tables = hw_specs.get_activation_tables(nc.m.arch)
    keep = "natural_log_exp_and_others"
    Exp_f = mybir.ActivationFunctionType.Exp
    Ln_f = mybir.ActivationFunctionType.Ln
    for name, fns in tables.items():
        if name != keep:
            fns.discard(Exp_f)
            fns.discard(Ln_f)

    xT = nc.dram_tensor("xT", [DIM, N], f32, kind="ExternalInput")
    wq = nc.dram_tensor("wq", [DIM, GDIM], f32, kind="ExternalInput")
    wk = nc.dram_tensor("wk", [DIM, GDIM], f32, kind="ExternalInput")
    wv = nc.dram_tensor("wv", [DIM, GDIM], f32, kind="ExternalInput")
    wo = nc.dram_tensor("wo", [DIM, GDIM], f32, kind="ExternalInput")
    outT = nc.dram_tensor("outT", [GDIM, N], f32, kind="ExternalOutput")

    with tile.TileContext(nc) as tc:
        with (
            tc.tile_pool(name="stage", bufs=3) as stage,    # fp32 staging
            tc.tile_pool(name="io", bufs=KO) as io,         # xc / ag chunks
            tc.tile_pool(name="wpool", bufs=1) as wpool,    # weights
            tc.tile_pool(name="qkpool", bufs=1) as qkpool,  # QT/KT/V
            tc.tile_pool(name="ptpool", bufs=6) as ptpool,  # exp(S^T)
            tc.tile_pool(name="work", bufs=4) as work,      # small staging
            tc.tile_pool(name="psS", bufs=2, space="PSUM") as psS,  # 2x2 banks
            tc.tile_pool(name="psO", bufs=4, space="PSUM") as psO,  # 4x1 banks
            tc.tile_pool(name="dram", bufs=1, space="DRAM") as dram,
        ):
            # ---- load weights fp32, cast to bf16 ----
            w_bf = {}
            for idx, (nm, t_dram) in enumerate(
                (("wq", wq), ("wk", wk), ("wv", wv), ("wo", wo))
            ):
                st = stage.tile([P, KO, GDIM], f32, tag="stage", name=f"st_{nm}")
                nc.sync.dma_start(st[:], t_dram.rearrange("(ko p) f -> p ko f", p=P))
                wbf = wpool.tile([P, KO, GDIM], bf16, name=f"bf_{nm}")
                if idx % 2 == 0:
                    nc.vector.tensor_copy(wbf[:], st[:])
                else:
                    nc.scalar.copy(wbf[:], st[:])
                w_bf[nm] = wbf
            wq_bf, wk_bf, wv_bf, wo_bf = w_bf["wq"], w_bf["wk"], w_bf["wv"], w_bf["wo"]

            # ---- load xT chunks fp32, cast to bf16 ----
            xc = []
            for k in range(KO):
                st = stage.tile([P, N], f32, tag="stage", name=f"stx{k}")
                nc.sync.dma_start(st[:], xT[k * P : (k + 1) * P, :])
                xck = io.tile([P, N], bf16, tag="io", name=f"xc{k}")
                if k % 3 == 0:
                    nc.vector.tensor_copy(xck[:], st[:])
                elif k % 3 == 1:
                    nc.scalar.copy(xck[:], st[:])
                else:
                    nc.gpsimd.tensor_copy(xck[:], st[:])
                xc.append(xck)


            # ---- Q^T, K^T projections -> bf16 [128f, 2, 2048] (Q pre-scaled) --
            QT = qkpool.tile([P, 2, N], bf16)
            KT = qkpool.tile([P, 2, N], bf16)
            for ni in range(NB):
                nsl = slice(ni * QB, (ni + 1) * QB)
                for fi in range(2):
                    pq = psS.tile([P, 1024], f32, tag="S", name="pq")
                    for k in range(KO):
                        nc.tensor.matmul(
                            pq[:, :QB],
                            wq_bf[:, k, fi * P : (fi + 1) * P],
                            xc[k][:, nsl],
                            start=(k == 0),
                            stop=(k == KO - 1),
                        )
                    # copy out with the softmax scale folded into Q
                    nc.scalar.mul(QT[:, fi, nsl], pq[:, :QB], SCALE)
                    pk = psS.tile([P, 1024], f32, tag="S", name="pk")
                    for k in range(KO):
                        nc.tensor.matmul(
                            pk[:, :QB],
                            wk_bf[:, k, fi * P : (fi + 1) * P],
                            xc[k][:, nsl],
                            start=(k == 0),
                            stop=(k == KO - 1),
                        )
                    nc.vector.tensor_copy(KT[:, fi, nsl], pk[:, :QB])

            # ---- V (natural layout) + ones column: [128j, 16, 4, 65] bf16 ----
            V_sb = qkpool.tile([P, JT, GROUPS, DIM_HEAD + 1], bf16)
            nc.vector.memset(V_sb[:, :, :, DIM_HEAD : DIM_HEAD + 1], 1.0)
            for jt in range(JT):
                pvt = psS.tile([P, 1024], f32, tag="S", name="pv")
                pv = pvt[:, :GDIM]
                for k in range(KO):
                    nc.tensor.matmul(
                        pv,
                        xc[k][:, jt * P : (jt + 1) * P],
                        wv_bf[:, k, :],
                        start=(k == 0),
                        stop=(k == KO - 1),
                    )
                nc.vector.tensor_copy(
                    V_sb[:, jt, :, 0:DIM_HEAD],
                    pv.rearrange("p (h d) -> p h d", h=GROUPS),
                )

            # ---- attention (mb-outer so AllGather chunks overlap compute) ----
            ag_outs = []
            for mb in range(NB):
                qsl = slice(mb * QB, (mb + 1) * QB)
                njc = 4 * (mb + 1)
                # one [65, 512] accumulator per head
                po = [
                    psO.tile([DIM_HEAD + 1, QB], f32, tag="O", name=f"po{h}")
                    for h in range(GROUPS)
                ]
                for jc in range(njc):
                    jsl = slice(jc * P, (jc + 1) * P)
                    t = jc - 4 * mb  # >= 0 on the diagonal 512-block
                    for hp in range(2):
                        ps = psS.tile([P, 1024], f32, tag="S", name=f"ps{hp}")
                        for s in range(2):
                            prow = slice(64 * s, 64 * s + 64)
                            nc.tensor.matmul(
                                ps[:, s * QB : (s + 1) * QB],
                                KT[prow, hp, jsl],
                                QT[prow, hp, qsl],
                                tile_position=(64 * s, 0),
                            )
                        pt = ptpool.tile([P, 1024], bf16, tag="pt", name="pt")
                        if t < 0:
                            nc.scalar.activation(pt[:], ps[:], Exp)
                        else:
                            c0 = t * P
                            ps3 = ps.rearrange("p (s q) -> p s q", s=2)
                            pt3 = pt.rearrange("p (s q) -> p s q", s=2)
                            if c0 > 0:
                                nc.vector.memset(pt3[:, :, :c0], 0.0)
                            nc.scalar.activation(pt3[:, :, c0:], ps3[:, :, c0:], Exp)
                            # causal: keep iff (q - j) >= 0  <=>  i1 - r >= 0
                            nc.gpsimd.affine_select(
                                out=pt3[:, :, c0:],
                                in_=pt3[:, :, c0:],
                                pattern=[[0, 2], [1, QB - c0]],
                                compare_op=mybir.AluOpType.is_ge,
                                fill=0.0,
                                base=0,
                                channel_multiplier=-1,
                            )
                        for s in range(2):
                            head = 2 * hp + s
                            nc.tensor.matmul(
                                po[head][:],
                                V_sb[:, jc, head, :],
                                pt[:, s * QB : (s + 1) * QB],
                                start=(jc == 0),
                                stop=(jc == njc - 1),
                            )
                # normalize + emit this q-chunk's AllGather
                attnT_mb = work.tile([DIM_HEAD, GROUPS, QB], bf16, tag="attnT")
                for head in range(GROUPS):
                    # 1/s = exp(-ln(s)) on ScalarE (same act table set as the
                    # attention exp; DVE reciprocal costs 3.3us per call)
                    lntmp = work.tile([1, QB], f32, tag="lntmp", name="lntmp")
                    nc.scalar.activation(
                        lntmp[:],
                        po[head][DIM_HEAD : DIM_HEAD + 1, :],
                        mybir.ActivationFunctionType.Ln,
                    )
                    recip = work.tile([1, QB], f32, tag="recip", name="recip")
                    nc.scalar.activation(
                        recip[:], lntmp[:], mybir.ActivationFunctionType.Exp,
                        scale=-1.0,
                    )
                    bc = work.tile([DIM_HEAD, QB], f32, tag="bc", name="bc")
                    nc.gpsimd.partition_broadcast(bc[:], recip[:])
                    nc.vector.tensor_mul(
                        attnT_mb[:, head, :], po[head][0:DIM_HEAD, :], bc[:]
                    )
                ag_in = dram.tile(
                    [GDIM, QB], bf16, name=f"ag_in{mb}", tag=f"ag_in{mb}"
                )
                ag_out = dram.tile(
                    [DIM, QB], bf16, name=f"ag_out{mb}", tag=f"ag_out{mb}"
                )
                nc.sync.dma_start(
                    ag_in.rearrange("(h p) q -> p h q", p=DIM_HEAD), attnT_mb[:]
                )
                nc.gpsimd.collective_compute(
                    "AllGather",
                    mybir.AluOpType.bypass,
                    replica_groups=[[0, 1, 2, 3], [4, 5, 6, 7]],
                    ins=[ag_in.opt()],
                    outs=[ag_out.opt()],
                )
                ag_outs.append(ag_out)

            # ---- output projection (Wo column slice), per q-chunk ----
            for mb in range(NB):
                qsl = slice(mb * QB, (mb + 1) * QB)
                agb = []
                for k in range(KO):
                    agbk = io.tile([P, N], bf16, tag="io", name=f"agb{k}")
                    nc.sync.dma_start(
                        agbk[:, :QB], ag_outs[mb][k * P : (k + 1) * P, :]
                    )
                    agb.append(agbk)
                for fi in range(2):
                    pw = psS.tile([P, 1024], f32, tag="S", name="pw")
                    for k in range(KO):
                        nc.tensor.matmul(
                            pw[:, :QB],
                            wo_bf[:, k, fi * P : (fi + 1) * P],
                            agb[k][:, :QB],
                            start=(k == 0),
                            stop=(k == KO - 1),
                        )
                    ot = work.tile([P, QB], f32, tag="ot", name="ot")
                    nc.vector.tensor_copy(ot[:], pw[:, :QB])
                    nc.sync.dma_start(outT[fi * P : (fi + 1) * P, qsl], ot[:])

    nc.finalize()
    return nc


def _get_nc():
    global _cached
    if _cached is None:
        _cached = _build_nc()
    return _cached


def _m0_const():
    import ml_dtypes

    m = (np.arange(QB)[None, :] >= np.arange(P)[:, None]).astype(np.float32)
    return m.astype(ml_dtypes.bfloat16)


def kernel(x, mask, Wq, Wk, Wv, Wo):
    x = np.asarray(x, dtype=np.float32)
    mask = np.asarray(mask)
    Wq = np.asarray(Wq, dtype=np.float32)
    Wk = np.asarray(Wk, dtype=np.float32)
    Wv = np.asarray(Wv, dtype=np.float32)
    Wo = np.asarray(Wo, dtype=np.float32)
    # this problem's padding mask is all-True (spec fill: ones); the kernel
    # relies on that (only the causal mask is applied on device).
    assert mask.all(), "kernel specialized for all-ones padding mask"

    from concourse import bass_utils

    nc = _get_nc()

    xTs = [np.ascontiguousarray(x[b].T) for b in range(B)]
    in_maps = []
    for c in range(NCORES):
        b, g = divmod(c, GROUPS)
        gsl = slice(g * GDIM, (g + 1) * GDIM)
        in_maps.append(
            {
                "xT": xTs[b],
                "wq": np.ascontiguousarray(Wq[:, gsl]),
                "wk": np.ascontiguousarray(Wk[:, gsl]),
                "wv": np.ascontiguousarray(Wv[:, gsl]),
                "wo": np.ascontiguousarray(Wo[:, gsl]),
            }
        )

    res = bass_utils.run_bass_kernel_spmd(nc, in_maps, core_ids=list(range(NCORES)))

    out = np.empty((B, N, DIM), dtype=np.float32)
    for c in range(NCORES):
        b, g = divmod(c, GROUPS)
        out[b, :, g * GDIM : (g + 1) * GDIM] = res.results[c]["outT"].T
    return out
